# revision 5
# baseline (speedup 1.0000x reference)
"""Multi-head attention (B=2, L=2048, D=1024, H=16) on 8 TRN2 NeuronCores.

Sharding: 2 batches x 4 head-groups (4 heads each). Core c handles batch
c//4, heads [4*(c%4), 4*(c%4)+4). Each core computes its Q/K/V projections
(column-sharded weights), attention for its 4 heads, and a row-sharded
partial of the output projection. The host sums the 4 partials per batch
(the Wo all-reduce) and folds in b_o and the b_v contribution (softmax rows
sum to 1, so b_v's effect on the output is the constant row b_v @ w_o.T).

On-device layouts (per core, host pre-transposes so no device transposes):
  xqT/xkT/xvT [D, L]   activations, feature-major
  wqT/wkT/wvT [D, F]   W[S,:].T for this core's F=256 head features
  woR        [F, D]    w_o[:, S].T
  qpT/kpT    [F, L]    projected Q/K, head-feature-major (= Wq_s @ Xq.T)
  vp         [L, F+4]  projected V with a ones column per head (the ones
                       column makes the AV matmul emit softmax denominators
                       as an extra output row)
  expT       [L, L]    exp(scale * K Q^T) tiles, key-major so softmax's sum
                       reduction and the AV contraction are both over keys
All matmul operands are float32r (FP22) so the PE runs at full rate; PSUM
accumulation stays fp32. Softmax skips the max subtraction (energy*scale is
bounded by ~±3 for these input scales).
"""

import numpy as np

import concourse.bass as bass
import concourse.mybir as mybir
import concourse.tile as tile
from concourse import bacc
from concourse import bass_utils

F32 = mybir.dt.float32
F32R = mybir.dt.float32r
ACT = mybir.ActivationFunctionType

B = 2
L = 2048
D = 1024
HEADS = 16
DH = 64
N_CORES = 8
GROUPS = 4                 # head groups (tensor-parallel dimension)
HG = HEADS // GROUPS       # heads per core
F = HG * DH                # head features per core (256)


def build_program(seq_len=L, d_model=D, hg=HG, dh=DH, uq=1024, wk_bufs=20,
                  mm_bufs=3):
    """Build the single-core Bass program (same program on all 8 cores)."""
    f = hg * dh                       # per-core head features
    kt_n = d_model // 128             # contraction tiles for projections
    lt_n = seq_len // 128             # sequence partition tiles
    mt_n = f // 128                   # head-feature partition tiles
    uq = min(uq, seq_len)
    un_n = seq_len // uq              # q-chunks ("units") per head
    ns_n = uq // 512                  # 512-wide matmul slices per unit
    scale = 1.0 / float(np.sqrt(dh))

    nc = bacc.Bacc("TRN2", target_bir_lowering=False, debug=False,
                   num_devices=N_CORES)

    xqT = nc.dram_tensor("xqT", [d_model, seq_len], F32, kind="ExternalInput").ap()
    xkT = nc.dram_tensor("xkT", [d_model, seq_len], F32, kind="ExternalInput").ap()
    xvT = nc.dram_tensor("xvT", [d_model, seq_len], F32, kind="ExternalInput").ap()
    wqT = nc.dram_tensor("wqT", [d_model, f], F32, kind="ExternalInput").ap()
    wkT = nc.dram_tensor("wkT", [d_model, f], F32, kind="ExternalInput").ap()
    wvT = nc.dram_tensor("wvT", [d_model, f], F32, kind="ExternalInput").ap()
    woR = nc.dram_tensor("woR", [f, d_model], F32, kind="ExternalInput").ap()
    bq = nc.dram_tensor("bq", [mt_n, 128, 1], F32, kind="ExternalInput").ap()
    bk = nc.dram_tensor("bk", [mt_n, 128, 1], F32, kind="ExternalInput").ap()
    out = nc.dram_tensor("out", [seq_len, d_model], F32, kind="ExternalOutput").ap()

    ones_dram = nc.inline_tensor(
        np.ones((128, hg * (dh + 1)), np.float32), name="ones_const").ap()

    r32 = lambda ap: ap.bitcast(F32R)

    with tile.TileContext(nc) as tc:
        with (
            tc.tile_pool(name="persist", bufs=1) as pp,
            tc.tile_pool(name="work", bufs=wk_bufs) as wp,
            tc.tile_pool(name="psmm", bufs=mm_bufs, space="PSUM") as pmm,
            tc.tile_pool(name="psav", bufs=1, space="PSUM") as pav,
        ):
            dma = nc.sync

            # ---- persistent tiles (f32r: all are matmul operands) -----
            wq_sb = pp.tile([128, kt_n * f], F32R, tag="wq", name="wq")
            wk_sb = pp.tile([128, kt_n * f], F32R, tag="wk", name="wk")
            wv_sb = pp.tile([128, kt_n * f], F32R, tag="wv", name="wv")
            wo_sb = [pp.tile([128, d_model], F32R, tag=f"wo{i}", name=f"wo{i}")
                     for i in range(mt_n)]
            qpT = [pp.tile([128, seq_len], F32R, tag=f"qpT{i}", name=f"qpT{i}")
                   for i in range(mt_n)]
            kpT = [pp.tile([128, seq_len], F32R, tag=f"kpT{i}", name=f"kpT{i}")
                   for i in range(mt_n)]
            ctxT = [pp.tile([128, seq_len], F32R, tag=f"ctxT{i}", name=f"ctxT{i}")
                    for i in range(mt_n)]
            vp = [pp.tile([128, hg * (dh + 1)], F32R, tag=f"vp{i}", name=f"vp{i}")
                  for i in range(lt_n)]
            bq_sb = [pp.tile([128, 1], F32, tag=f"bq{i}", name=f"bq{i}")
                     for i in range(mt_n)]
            bk_sb = [pp.tile([128, 1], F32, tag=f"bk{i}", name=f"bk{i}")
                     for i in range(mt_n)]
            ones_sb = pp.tile([1, dh], F32R, tag="ones", name="ones")

            # ---- weight / bias loads ----------------------------------
            for k in range(kt_n):
                ksl = slice(k * 128, (k + 1) * 128)
                dma.dma_start(wq_sb[:, k * f:(k + 1) * f], r32(wqT[ksl, :]))
                dma.dma_start(wk_sb[:, k * f:(k + 1) * f], r32(wkT[ksl, :]))
                dma.dma_start(wv_sb[:, k * f:(k + 1) * f], r32(wvT[ksl, :]))
            for i in range(mt_n):
                dma.dma_start(wo_sb[i][:], r32(woR[i * 128:(i + 1) * 128, :]))
                dma.dma_start(bq_sb[i][:], bq[i])
                dma.dma_start(bk_sb[i][:], bk[i])
            dma.dma_start(ones_sb[:], r32(ones_dram[0:1, 0:dh]))

            # ---- Q / K projections: qpT = Wq_s @ Xq.T -----------------
            for xT, w_sb, dstT, b_sb in ((xqT, wq_sb, qpT, bq_sb),
                                         (xkT, wk_sb, kpT, bk_sb)):
                for u in range(un_n):
                    usl = slice(u * uq, (u + 1) * uq)
                    xt = []
                    for k in range(kt_n):
                        t = wp.tile([128, uq], F32R, tag="wkt", name="wkt")
                        dma.dma_start(t[:], r32(xT[k * 128:(k + 1) * 128, usl]))
                        xt.append(t)
                    for m in range(mt_n):
                        ps = pmm.tile([128, uq], F32, tag="mm", name="mm")
                        for ns in range(ns_n):
                            nsl = slice(ns * 512, (ns + 1) * 512)
                            for k in range(kt_n):
                                nc.tensor.matmul(
                                    ps[:, nsl],
                                    w_sb[:, k * f + m * 128:k * f + (m + 1) * 128],
                                    xt[k][:, nsl],
                                    start=(k == 0), stop=(k == kt_n - 1))
                        nc.scalar.activation(dstT[m][:, usl], ps[:], ACT.Identity,
                                             bias=b_sb[m][:])

            # ---- V projection: vp = Xv @ Wv_s.T (natural layout) ------
            lt_per_u = uq // 128
            for u in range(un_n):
                usl = slice(u * uq, (u + 1) * uq)
                xt = []
                for k in range(kt_n):
                    t = wp.tile([128, uq], F32R, tag="wkt", name="wkt")
                    dma.dma_start(t[:], r32(xvT[k * 128:(k + 1) * 128, usl]))
                    xt.append(t)
                for j in range(lt_per_u):
                    m = u * lt_per_u + j
                    ps = pmm.tile([128, f], F32, tag="mm", name="mm")
                    for k in range(kt_n):
                        nc.tensor.matmul(
                            ps[:],
                            xt[k][:, j * 128:(j + 1) * 128],
                            wv_sb[:, k * f:(k + 1) * f],
                            start=(k == 0), stop=(k == kt_n - 1))
                    nc.sync.dma_start(
                        vp[m][:].rearrange("p (h e) -> p h e", e=dh + 1)[:, :, dh:dh + 1],
                        r32(ones_dram[:, 0:hg].rearrange("p (h o) -> p h o", o=1)))
                    nc.vector.tensor_copy(
                        vp[m][:].rearrange("p (h e) -> p h e", e=dh + 1)[:, :, 0:dh],
                        ps[:].rearrange("p (h d) -> p h d", d=dh))

            # ---- attention per head -----------------------------------
            for h in range(hg):
                mt, off = divmod(h * dh, 128)
                hsl = slice(off, off + dh)
                qh = qpT[mt][hsl, :]
                kh = kpT[mt][hsl, :]
                for u in range(un_n):
                    av = pav.tile([dh + 1, uq], F32, tag="av", name="av")
                    for kt in range(lt_n):
                        eps = pmm.tile([128, uq], F32, tag="mm", name="mm")
                        for ns in range(ns_n):
                            nsl = slice(ns * 512, (ns + 1) * 512)
                            nc.tensor.matmul(
                                eps[:, nsl],
                                kh[:, kt * 128:(kt + 1) * 128],
                                qh[:, u * uq + ns * 512:u * uq + (ns + 1) * 512],
                                start=True, stop=True)
                        ex = wp.tile([128, uq], F32R, tag="wkt", name="wkt")
                        nc.scalar.activation(ex[:], eps[:], ACT.Exp, scale=scale)
                        for ns in range(ns_n):
                            nsl = slice(ns * 512, (ns + 1) * 512)
                            nc.tensor.matmul(
                                av[:, nsl],
                                vp[kt][:, h * (dh + 1):(h + 1) * (dh + 1)],
                                ex[:, nsl],
                                start=(kt == 0), stop=(kt == lt_n - 1))
                    # normalize: ctxT = av[0:dh] * (1 / av[dh]) broadcast
                    for ns in range(ns_n):
                        nsl = slice(ns * 512, (ns + 1) * 512)
                        csl = slice(u * uq + ns * 512, u * uq + (ns + 1) * 512)
                        s_sb = wp.tile([1, 512], F32R, tag="r", bufs=2, name="r")
                        nc.vector.tensor_copy(s_sb[:], av[dh:dh + 1, nsl])
                        bc = pmm.tile([dh, 512], F32, tag="mm", name="mm")
                        nc.tensor.matmul(bc[:], ones_sb[:], s_sb[:],
                                         start=True, stop=True)
                        rb = wp.tile([dh, 512], F32, tag="rb", bufs=2, name="rb")
                        nc.vector.reciprocal_approx_fast(out=rb[:], in_=bc[:])
                        nc.vector.tensor_mul(ctxT[mt][hsl, csl], av[0:dh, nsl], rb[:])

            # ---- output projection: out = ctxT.T @ woR ----------------
            for qt in range(lt_n):
                qsl = slice(qt * 128, (qt + 1) * 128)
                ps = pmm.tile([128, d_model], F32, tag="mm", name="mm")
                for ns in range(d_model // 512):
                    nsl = slice(ns * 512, (ns + 1) * 512)
                    for kc in range(mt_n):
                        nc.tensor.matmul(
                            ps[:, nsl],
                            ctxT[kc][:, qsl],
                            wo_sb[kc][:, nsl],
                            start=(kc == 0), stop=(kc == mt_n - 1))
                ob = wp.tile([128, d_model], F32, tag="wkt", name="wkt")
                nc.vector.tensor_copy(ob[:], ps[:])
                dma.dma_start(out[qsl, :], ob[:])

    nc.compile()
    return nc


def make_in_maps(q, k, v, w_q, w_k, w_v, w_o, b_q, b_k):
    """Per-core input maps for the 8-way (batch x head-group) sharding."""
    f32 = lambda a: np.ascontiguousarray(np.asarray(a, dtype=np.float32))
    in_maps = []
    for c in range(N_CORES):
        b, g = divmod(c, GROUPS)
        S = slice(g * F, (g + 1) * F)
        in_maps.append({
            "xqT": f32(np.asarray(q)[b].T),
            "xkT": f32(np.asarray(k)[b].T),
            "xvT": f32(np.asarray(v)[b].T),
            "wqT": f32(np.asarray(w_q)[S, :].T),
            "wkT": f32(np.asarray(w_k)[S, :].T),
            "wvT": f32(np.asarray(w_v)[S, :].T),
            "woR": f32(np.asarray(w_o)[:, S].T),
            "bq": f32(np.asarray(b_q)[S].reshape(F // 128, 128, 1)),
            "bk": f32(np.asarray(b_k)[S].reshape(F // 128, 128, 1)),
        })
    return in_maps


_PROGRAM = None


def _get_program():
    global _PROGRAM
    if _PROGRAM is None:
        _PROGRAM = build_program()
    return _PROGRAM


def run_on_hw(in_maps, trace=False, **kwargs):
    nc = _get_program()
    return bass_utils.run_bass_kernel_spmd(
        nc, in_maps, core_ids=list(range(N_CORES)), trace=trace, **kwargs)


def kernel(q, k, v, w_q, b_q, w_k, b_k, w_v, b_v, w_o, b_o):
    q, k, v = (np.asarray(a, np.float32) for a in (q, k, v))
    w_o = np.asarray(w_o, np.float32)
    in_maps = make_in_maps(q, k, v, w_q, w_k, w_v, w_o, b_q, b_k)
    res = run_on_hw(in_maps)
    outs = [r["out"] for r in res.results]
    # host-side gather: sum head-group partials, fold b_o and b_v terms
    const_row = (np.asarray(b_v, np.float32) @ w_o.T
                 + np.asarray(b_o, np.float32)).astype(np.float32)
    full = np.empty((B, L, D), np.float32)
    for b in range(B):
        full[b] = outs[GROUPS * b]
        for g in range(1, GROUPS):
            full[b] += outs[GROUPS * b + g]
        full[b] += const_row
    return full


# revision 14
# speedup vs baseline: 3.2191x; 3.2191x over previous
"""Multi-head attention (B=2, L=2048, D=1024, H=16) on 8 TRN2 NeuronCores.

Sharding: 2 batches x 4 head-groups (4 heads each). Core c handles batch
c//4, heads [4*(c%4), 4*(c%4)+4). Each core computes its Q/K/V projections
(column-sharded weights), attention for its 4 heads, and a row-sharded
partial of the output projection. The host sums the 4 partials per batch
(the Wo all-reduce) and folds in b_o and the b_v contribution (softmax rows
sum to 1, so b_v's effect on the output is the constant row b_v @ w_o.T).

On-device layouts (per core, host pre-transposes/chunks so every DMA is a
single contiguous block and no device transposes are needed):
  xqT/xkT/xvT [U, D, uq] activations, feature-major, pre-chunked into U
                         sequence chunks of uq so [128, uq] k-tiles are
                         contiguous in DRAM
  wqT/wkT/wvT [D, F]     W[S,:].T for this core's F=256 head features
  woR        [F, D]      w_o[:, S].T
  qpT/kpT    [F, L]      projected Q/K, head-feature-major (= Wq_s @ Xq.T)
  vp         [L, F+4]    projected V with a ones column per head (the ones
                         column makes the AV matmul emit softmax denominators
                         as an extra output row)
  expT       [L, L]      exp(scale * K Q^T) tiles, key-major so softmax's sum
                         reduction and the AV contraction are both over keys
All matmul operands are float32r (FP22) so the PE runs at full rate; PSUM
accumulation stays fp32. Softmax skips the max subtraction (energy*scale is
bounded by ~±3 for these input scales).
"""

import numpy as np

import concourse.mybir as mybir
import concourse.tile as tile
from concourse import bacc
from concourse import bass_utils

F32 = mybir.dt.float32
F32R = mybir.dt.float32r
ACT = mybir.ActivationFunctionType

B = 2
L = 2048
D = 1024
HEADS = 16
DH = 64
N_CORES = 8
GROUPS = 4                 # head groups (tensor-parallel dimension)
HG = HEADS // GROUPS       # heads per core
F = HG * DH                # head features per core (256)
UQ = 1024                  # q-chunk ("unit") size


def build_program(seq_len=L, d_model=D, hg=HG, dh=DH, uq=UQ, wk_bufs=20,
                  mm_bufs=2, replicas=1):
    """Build the single-core Bass program (same program on all 8 cores)."""
    f = hg * dh                       # per-core head features
    kt_n = d_model // 128             # contraction tiles for projections
    lt_n = seq_len // 128             # sequence partition tiles
    mt_n = f // 128                   # head-feature partition tiles
    uq = min(uq, seq_len)
    un_n = seq_len // uq              # q-chunks ("units") per head
    ns_n = uq // 512                  # 512-wide matmul slices per unit
    scale = 1.0 / float(np.sqrt(dh))

    nc = bacc.Bacc("TRN2", target_bir_lowering=False, debug=False,
                   num_devices=N_CORES)

    xqT = nc.dram_tensor("xqT", [un_n, d_model, uq], F32, kind="ExternalInput").ap()
    xkT = nc.dram_tensor("xkT", [un_n, d_model, uq], F32, kind="ExternalInput").ap()
    xvT = nc.dram_tensor("xvT", [un_n, d_model, uq], F32, kind="ExternalInput").ap()
    wqT = nc.dram_tensor("wqT", [d_model, f], F32, kind="ExternalInput").ap()
    wkT = nc.dram_tensor("wkT", [d_model, f], F32, kind="ExternalInput").ap()
    wvT = nc.dram_tensor("wvT", [d_model, f], F32, kind="ExternalInput").ap()
    woR = nc.dram_tensor("woR", [f, d_model], F32, kind="ExternalInput").ap()
    bq = nc.dram_tensor("bq", [mt_n, 128, 1], F32, kind="ExternalInput").ap()
    bk = nc.dram_tensor("bk", [mt_n, 128, 1], F32, kind="ExternalInput").ap()
    out = nc.dram_tensor("out", [seq_len, d_model], F32, kind="ExternalOutput").ap()

    r32 = lambda ap: ap.bitcast(F32R)

    with tile.TileContext(nc) as tc:
        with (
            tc.tile_pool(name="persist", bufs=1) as pp,
            tc.tile_pool(name="work", bufs=wk_bufs) as wp,
            tc.tile_pool(name="psmm", bufs=mm_bufs, space="PSUM") as pmm,
            tc.tile_pool(name="psav", bufs=1, space="PSUM") as pav,
            tc.tile_pool(name="psbc", bufs=2, space="PSUM") as pbc,
        ):
            dma = nc.sync

            # ---- persistent tiles (f32r: all are matmul operands) -----
            wq_sb = pp.tile([128, kt_n * f], F32R, tag="wq", name="wq")
            wk_sb = pp.tile([128, kt_n * f], F32R, tag="wk", name="wk")
            wv_sb = pp.tile([128, kt_n * f], F32R, tag="wv", name="wv")
            wo_sb = [pp.tile([128, d_model], F32R, tag=f"wo{i}", name=f"wo{i}")
                     for i in range(mt_n)]
            qpT = [pp.tile([128, seq_len], F32R, tag=f"qpT{i}", name=f"qpT{i}")
                   for i in range(mt_n)]
            kpT = [pp.tile([128, seq_len], F32R, tag=f"kpT{i}", name=f"kpT{i}")
                   for i in range(mt_n)]
            ctxT = [pp.tile([128, seq_len], F32R, tag=f"ctxT{i}", name=f"ctxT{i}")
                    for i in range(mt_n)]
            vp = [pp.tile([128, hg * (dh + 1)], F32R, tag=f"vp{i}", name=f"vp{i}")
                  for i in range(lt_n)]
            bq_sb = [pp.tile([128, 1], F32, tag=f"bq{i}", name=f"bq{i}")
                     for i in range(mt_n)]
            bk_sb = [pp.tile([128, 1], F32, tag=f"bk{i}", name=f"bk{i}")
                     for i in range(mt_n)]
            ones_sb = pp.tile([1, dh], F32R, tag="ones", name="ones")
            ones4 = pp.tile([128, dh], F32, tag="ones4", name="ones4")

            # ---- weight / bias / const loads --------------------------
            for k in range(kt_n):
                ksl = slice(k * 128, (k + 1) * 128)
                dma.dma_start(wk_sb[:, k * f:(k + 1) * f], r32(wkT[ksl, :]))
                dma.dma_start(wq_sb[:, k * f:(k + 1) * f], r32(wqT[ksl, :]))
                dma.dma_start(wv_sb[:, k * f:(k + 1) * f], r32(wvT[ksl, :]))
            for i in range(mt_n):
                dma.dma_start(wo_sb[i][:], r32(woR[i * 128:(i + 1) * 128, :]))
                dma.dma_start(bq_sb[i][:], bq[i])
                dma.dma_start(bk_sb[i][:], bk[i])
            nc.gpsimd.memset(ones4[:], 1.0)
            nc.vector.tensor_copy(ones_sb[:], ones4[0:1, :])

            def project_qk(xT, w_sb, dstT, b_sb, u):
                """dstT[:, u-chunk] = W_s @ X.T + b (transposed projection)."""
                usl = slice(u * uq, (u + 1) * uq)
                xt = []
                for k in range(kt_n):
                    t = wp.tile([128, uq], F32R, tag="wkt", name="wkt")
                    dma.dma_start(t[:], r32(xT[u, k * 128:(k + 1) * 128, :]))
                    xt.append(t)
                for m in range(mt_n):
                    ps = pmm.tile([128, uq], F32, tag="mm", name="mm")
                    for ns in range(ns_n):
                        nsl = slice(ns * 512, (ns + 1) * 512)
                        for k in range(kt_n):
                            nc.tensor.matmul(
                                ps[:, nsl],
                                w_sb[:, k * f + m * 128:k * f + (m + 1) * 128],
                                xt[k][:, nsl],
                                start=(k == 0), stop=(k == kt_n - 1))
                    nc.vector.tensor_scalar_add(dstT[m][:, usl], ps[:], b_sb[m][:])

            def project_v(u):
                """vp rows for unit u = Xv @ Wv_s.T, plus per-head ones cols."""
                xt = []
                for k in range(kt_n):
                    t = wp.tile([128, uq], F32R, tag="wkt", name="wkt")
                    dma.dma_start(t[:], r32(xvT[u, k * 128:(k + 1) * 128, :]))
                    xt.append(t)
                for j in range(uq // 128):
                    m = u * (uq // 128) + j
                    ps = pmm.tile([128, f], F32, tag="mm", name="mm")
                    for k in range(kt_n):
                        nc.tensor.matmul(
                            ps[:],
                            xt[k][:, j * 128:(j + 1) * 128],
                            wv_sb[:, k * f:(k + 1) * f],
                            start=(k == 0), stop=(k == kt_n - 1))
                    vpv = vp[m][:].rearrange("p (h e) -> p h e", e=dh + 1)
                    nc.vector.tensor_copy(
                        vpv[:, :, 0:dh],
                        ps[:].rearrange("p (h d) -> p h d", d=dh))
                    nc.vector.tensor_copy(
                        vpv[:, :, dh:dh + 1],
                        ones4[:, 0:hg].rearrange("p (h o) -> p h o", o=1))

            def attend(h, u):
                """One head x one q-chunk: energyT -> exp -> AV -> normalize.

                AV accumulation is interleaved per k-tile so each expT tile
                is consumed (and its slot freed) right after it is produced.
                """
                mt, off = divmod(h * dh, 128)
                hsl = slice(off, off + dh)
                qh = qpT[mt][hsl, :]
                kh = kpT[mt][hsl, :]
                av = pav.tile([dh + 1, uq], F32, tag="av", name="av")
                for kt in range(lt_n):
                    eps = pmm.tile([128, uq], F32, tag="mm", name="mm")
                    for ns in range(ns_n):
                        nsl = slice(ns * 512, (ns + 1) * 512)
                        nc.tensor.matmul(
                            eps[:, nsl],
                            kh[:, kt * 128:(kt + 1) * 128],
                            qh[:, u * uq + ns * 512:u * uq + (ns + 1) * 512],
                            start=True, stop=True)
                    ex = wp.tile([128, uq], F32R, tag="wkt", name="wkt")
                    nc.scalar.activation(ex[:], eps[:], ACT.Exp, scale=scale)
                    for ns in range(ns_n):
                        nsl = slice(ns * 512, (ns + 1) * 512)
                        nc.tensor.matmul(
                            av[:, nsl],
                            vp[kt][:, h * (dh + 1):(h + 1) * (dh + 1)],
                            ex[:, nsl],
                            start=(kt == 0), stop=(kt == lt_n - 1))
                for ns in range(ns_n):
                    nsl = slice(ns * 512, (ns + 1) * 512)
                    csl = slice(u * uq + ns * 512, u * uq + (ns + 1) * 512)
                    # normalize: ctxT = av[0:dh] * (1 / av[dh]) broadcast
                    s_sb = wp.tile([1, 512], F32R, tag="r", bufs=2, name="r")
                    nc.vector.tensor_copy(s_sb[:], av[dh:dh + 1, nsl])
                    bc = pbc.tile([dh, 512], F32, tag="bc", name="bc")
                    nc.tensor.matmul(bc[:], ones_sb[:], s_sb[:],
                                     start=True, stop=True)
                    rb = wp.tile([dh, 512], F32, tag="rb", bufs=2, name="rb")
                    nc.vector.reciprocal(out=rb[:], in_=bc[:])
                    nc.vector.tensor_mul(ctxT[mt][hsl, csl], av[0:dh, nsl], rb[:])

            def out_project(qt):
                """out rows qt*128.. = ctxT.T @ woR (this core's partial)."""
                qsl = slice(qt * 128, (qt + 1) * 128)
                ps = pmm.tile([128, d_model], F32, tag="mm", name="mm")
                for ns in range(d_model // 512):
                    nsl = slice(ns * 512, (ns + 1) * 512)
                    for kc in range(mt_n):
                        nc.tensor.matmul(
                            ps[:, nsl],
                            ctxT[kc][:, qsl],
                            wo_sb[kc][:, nsl],
                            start=(kc == 0), stop=(kc == mt_n - 1))
                ob = wp.tile([128, d_model], F32, tag="wkt", name="wkt")
                nc.vector.tensor_copy(ob[:], ps[:])
                dma.dma_start(out[qsl, :], ob[:])

            # ---- schedule: K fully, then per-unit Q, V, attention -----
            for _rep in range(replicas):
                for u in range(un_n):
                    project_qk(xkT, wk_sb, kpT, bk_sb, u)
                for u in range(un_n):
                    project_qk(xqT, wq_sb, qpT, bq_sb, u)
                for u in range(un_n):
                    project_v(u)
                for u in range(un_n):
                    for h in range(hg):
                        attend(h, u)
                    for qt in range(u * (uq // 128), (u + 1) * (uq // 128)):
                        out_project(qt)

    nc.compile()
    return nc


def make_in_maps(q, k, v, w_q, w_k, w_v, w_o, b_q, b_k):
    """Per-core input maps for the 8-way (batch x head-group) sharding."""
    f32 = lambda a: np.ascontiguousarray(np.asarray(a, dtype=np.float32))
    un_n = L // UQ
    chunk = lambda xT: np.ascontiguousarray(
        np.asarray(xT, np.float32).reshape(D, un_n, UQ).transpose(1, 0, 2))
    in_maps = []
    for c in range(N_CORES):
        b, g = divmod(c, GROUPS)
        S = slice(g * F, (g + 1) * F)
        in_maps.append({
            "xqT": chunk(np.asarray(q)[b].T),
            "xkT": chunk(np.asarray(k)[b].T),
            "xvT": chunk(np.asarray(v)[b].T),
            "wqT": f32(np.asarray(w_q)[S, :].T),
            "wkT": f32(np.asarray(w_k)[S, :].T),
            "wvT": f32(np.asarray(w_v)[S, :].T),
            "woR": f32(np.asarray(w_o)[:, S].T),
            "bq": f32(np.asarray(b_q)[S].reshape(F // 128, 128, 1)),
            "bk": f32(np.asarray(b_k)[S].reshape(F // 128, 128, 1)),
        })
    return in_maps


_PROGRAM = None


def _get_program():
    global _PROGRAM
    if _PROGRAM is None:
        _PROGRAM = build_program()
    return _PROGRAM


def run_on_hw(in_maps, trace=False, **kwargs):
    nc = _get_program()
    return bass_utils.run_bass_kernel_spmd(
        nc, in_maps, core_ids=list(range(N_CORES)), trace=trace, **kwargs)


def kernel(q, k, v, w_q, b_q, w_k, b_k, w_v, b_v, w_o, b_o):
    q, k, v = (np.asarray(a, np.float32) for a in (q, k, v))
    w_o = np.asarray(w_o, np.float32)
    in_maps = make_in_maps(q, k, v, w_q, w_k, w_v, w_o, b_q, b_k)
    res = run_on_hw(in_maps)
    outs = [r["out"] for r in res.results]
    # host-side gather: sum head-group partials, fold b_o and b_v terms
    const_row = (np.asarray(b_v, np.float32) @ w_o.T
                 + np.asarray(b_o, np.float32)).astype(np.float32)
    full = np.empty((B, L, D), np.float32)
    for b in range(B):
        full[b] = outs[GROUPS * b]
        for g in range(1, GROUPS):
            full[b] += outs[GROUPS * b + g]
        full[b] += const_row
    return full
